# revision 1
# baseline (speedup 1.0000x reference)
"""FFTTransformerBlock: full on-device Bass kernel, 8-core SPMD.

Sharding: data parallel over batch x row-slices (2 batches x 4 slices of 64
rows). Each core gets a zero-padded 68-row slice and computes the full block
(FSAS FFT-correlation attention + DFFN) locally; dwconv halos come from the
2 extra rows, FFT patches are 64 consecutive flattened pixels so they are
row-local, and image-edge zero padding is reproduced with per-core 0/1 mask
tiles so the single SPMD program serves all cores.

Layout: channels on partitions, flattened rows*W on the free axis. conv1x1 =
K-contraction matmul; depthwise 3x3 = 9 channel-diagonal matmuls accumulated
in PSUM over a 258-wide zero-padded row layout; rfft2/irfft2 correlation =
TensorE 128x128 transposes + real/imag 2D-DFT matmuls (block-diagonal 64x64
pairs); LayerNorm over channels = ones-matmul partition reduction broadcast +
Abs_reciprocal_sqrt activation. All SBUF data bf16, PSUM fp32.
"""

import sys
import time
import types

import numpy as np

sys.path.insert(0, "/opt/trn_rl_repo")

P = 8
EPS = 1e-5
B, C, H, W = 2, 64, 256, 256
HID = 2 * C          # 128
C6 = 6 * C           # 384
C2 = 2 * C           # 128
H2 = 2 * HID         # 256
NCORES = 8
RS = 64              # output rows per core
RX = RS + 4          # 68 rows incl 2+2 halo
NX = RX * W          # 17408
WP = W + 2           # 258 padded row width
MMN = 512            # matmul free-dim chunk

A_BLOCKS = [(0, 18), (18, 36), (36, 52), (52, 68)]   # x1/qkv row ranges
B_BLOCKS = [(2, 18), (18, 34), (34, 50), (50, 66)]   # output row ranges

_LAST_EXEC_NS = None
_LAST_RES = None
DEBUG_OUTS = False


# ---------------------------------------------------------------- host consts

def _bd(m):
    """64x64 -> 128x128 block diagonal."""
    z = np.zeros((128, 128), np.float32)
    z[:64, :64] = m
    z[64:, 64:] = m
    return z


def _dft_mats():
    idx = np.arange(64)
    a4, a5 = idx // 8, idx % 8
    ph = 2.0 * np.pi * (np.outer(a4, a4) + np.outer(a5, a5)) / 8.0
    # F[u,e] = exp(-i*2pi*(u4*a4+u5*a5)/8); symmetric
    u4 = a4[:, None] * a4[None, :] + a5[:, None] * a5[None, :]
    ang = 2.0 * np.pi * u4 / 8.0
    CR = np.cos(ang).astype(np.float32)
    CI = (-np.sin(ang)).astype(np.float32)
    return CR, CI


class _ConstPack:
    """Builds one [128, K] bf16 array; named column spans."""

    def __init__(self, bf16):
        self.cols = 0
        self.spans = {}
        self.chunks = []
        self.bf16 = bf16

    def add(self, name, arr, rows=None):
        arr = np.asarray(arr, np.float32)
        if arr.ndim == 1:
            arr = arr[:, None]
        r, c = arr.shape
        pad = np.zeros((128, c), np.float32)
        pad[:r, :] = arr
        self.spans[name] = (self.cols, c, r)
        self.chunks.append(pad)
        self.cols += c
        return name

    def finalize(self):
        full = np.concatenate(self.chunks, axis=1)
        return full.astype(self.bf16)


def _prepare_host(args, bf16):
    """Fold weights, build constant pack + per-core masks and xs slices."""
    ln1_g, ln1_b = args["ln1_g"], args["ln1_b"]
    ln2_g, ln2_b = args["ln2_g"], args["ln2_b"]

    # fold ln gains into following 1x1 convs
    Wh = args["att_hid_w"] * ln1_g[None, :]                      # [384, 64]
    bh = args["att_hid_b"] + args["att_hid_w"] @ ln1_b           # [384]
    Wf = args["ffn_in_w"] * ln2_g[None, :]                       # [256, 64]
    bf = args["ffn_in_b"] + args["ffn_in_w"] @ ln2_b             # [256]

    Wo = args["att_out_w"]                                       # [64, 128]
    bo = args["att_out_b"]
    Wob = Wo * args["att_norm_b"][None, :]                       # b-term of att_norm
    use_wob = bool(np.abs(Wob).max() > 0)

    W2 = args["ffn_out_w"]                                       # [64, 128]
    b2o = args["ffn_out_b"]

    # FFN spectral filter: require per-channel constant (scale) filter
    fft = args["ffn_fft"].reshape(H2, -1)
    s_ch = fft[:, 0].copy()
    if np.abs(fft - s_ch[:, None]).max() > 1e-6:
        raise NotImplementedError("non-constant ffn_fft needs spectral path")

    CR, CI = _dft_mats()

    cp = _ConstPack(bf16)
    cp.add("ones64", np.full((64, 64), 1.0 / 64.0))
    cp.add("ones128", np.full((128, 128), 1.0 / 128.0))
    cp.add("I128", np.eye(128))
    cp.add("CRbd", _bd(CR))
    cp.add("CIbd", _bd(CI))
    cp.add("CRibd", _bd(CR / 64.0))
    cp.add("CIibd", _bd(CI / 64.0))
    # folded conv1x1+dwconv taps (requires zero conv bias, checked here):
    # q_m = sum_dh,dw diag(wdw[:,dh,dw]) @ Wh_m @ shift(xn); taps (dw=0, dw=1)
    # K-packed into one K=128 matmul (upper xn copy pre-shifted), dw=2 single.
    if np.abs(bh).max() > 0 or np.abs(bf).max() > 0:
        raise NotImplementedError("folded dwconv path needs zero conv bias")
    wdw1 = args["att_dw_w"][:, 0]                                # [384, 3, 3]
    wdw2 = args["ffn_dw_w"][:, 0]                                # [256, 3, 3]
    for m in range(3):
        Whm = Wh[m * 128:(m + 1) * 128]                          # [128, 64]
        wm = wdw1[m * 128:(m + 1) * 128]
        for dh in range(3):
            st = np.zeros((128, 128), np.float32)
            st[:64] = (wm[:, dh, 0][:, None] * Whm).T
            st[64:] = (wm[:, dh, 1][:, None] * Whm).T
            cp.add(f"QW{m}{dh}", st)
            cp.add(f"QS{m}{dh}", (wm[:, dh, 2][:, None] * Whm).T)  # [64,128]
    cp.add("WoT", Wo.T)                                          # [128, 64]
    cp.add("WobT", Wob.T)
    for m in range(2):
        Wfm = Wf[m * 128:(m + 1) * 128] * s_ch[m * 128:(m + 1) * 128][:, None]
        wm = wdw2[m * 128:(m + 1) * 128]
        for dh in range(3):
            st = np.zeros((128, 128), np.float32)
            st[:64] = (wm[:, dh, 0][:, None] * Wfm).T
            st[64:] = (wm[:, dh, 1][:, None] * Wfm).T
            cp.add(f"YW{m}{dh}", st)
            cp.add(f"YS{m}{dh}", (wm[:, dh, 2][:, None] * Wfm).T)
    cp.add("W2T", W2.T)                                          # [128, 64]
    cst = cp.finalize()

    cs = _ConstPack(np.float32)
    cs.add("b_h", bh.reshape(3, 128).T)                          # [128, 3]
    cs.add("b_dw1", args["att_dw_b"].reshape(3, 128).T)          # [128, 3]
    cs.add("g2", args["att_norm_g"])                             # [128, 1]
    cs.add("b_o", bo)                                            # [64, 1]
    cs.add("b_f", bf.reshape(2, 128).T)                          # [128, 2]
    cs.add("s_ch", s_ch.reshape(2, 128).T)                       # [128, 2]
    cs.add("b_dw2", args["ffn_dw_b"].reshape(2, 128).T)          # [128, 2]
    cs.add("b2o", b2o)                                           # [64, 1]
    cs.add("eps", np.full(128, EPS))                             # [128, 1]
    cs32 = cs.finalize()

    # per-core xs slices + masks
    x = args["x"]                                                # [B, C, H, W]
    xs_list, msk_list = [], []
    for core in range(NCORES):
        bi, si = core // 4, core % 4
        g0 = 64 * si
        sl = np.zeros((C, RX, W), np.float32)
        lo, hi = g0 - 2, g0 + 66
        clo, chi = max(lo, 0), min(hi, H)
        sl[:, clo - lo:chi - lo, :] = x[bi, :, clo:chi, :]
        xs_list.append(sl.reshape(C, NX).astype(bf16))

        bot, top = si == 0, si == 3
        msk = np.ones((128, 2 * WP + 2 * WP + WP + WP), np.float32)
        # [hmask_lo(2*WP), hmask_hi(2*WP), ymask_lo(WP), ymask_hi(WP)]
        if bot:
            msk[:, 0:2 * WP] = 0.0            # h rows X0,X1 (global -2,-1)
            msk[:, 4 * WP:5 * WP] = 0.0       # y row X1 (global -1)
        if top:
            msk[:, 2 * WP:4 * WP] = 0.0       # h rows X66,X67
            msk[:, 5 * WP:6 * WP] = 0.0       # y row X66
        msk_list.append(msk.astype(bf16))

    zb = {
        "dw1": float(np.abs(args["att_dw_b"]).max()) == 0.0,
        "dw2": float(np.abs(args["ffn_dw_b"]).max()) == 0.0,
    }
    return (cst, cp.spans, cs32, cs.spans), xs_list, msk_list, use_wob, zb


# ---------------------------------------------------------------- device build


def _build(spans, cst_cols, spans32, cs32_cols, use_wob, zb, mybir, bacc, tile, bass):
    BF = mybir.dt.bfloat16
    F32 = mybir.dt.float32
    AF = mybir.ActivationFunctionType
    OP = mybir.AluOpType

    nc = bacc.Bacc("TRN2", target_bir_lowering=False, debug=False,
                   num_devices=NCORES)
    xs = nc.dram_tensor("xs", [C, NX], BF, kind="ExternalInput").ap()
    cstD = nc.dram_tensor("cst", [128, cst_cols], BF, kind="ExternalInput").ap()
    cs32D = nc.dram_tensor("cs32", [128, cs32_cols], F32, kind="ExternalInput").ap()
    mskD = nc.dram_tensor("msk", [128, 6 * WP], BF, kind="ExternalInput").ap()
    outD = nc.dram_tensor("out", [C, RS * W], F32, kind="ExternalOutput").ap()

    with tile.TileContext(nc) as tc:
        with (
            tc.tile_pool(name="persist", bufs=1) as pc,
            tc.tile_pool(name="blk", bufs=2) as pb,
            tc.tile_pool(name="chk", bufs=2) as pk,
            tc.tile_pool(name="io", bufs=2) as pio,
            tc.tile_pool(name="ps", bufs=2, space="PSUM") as pps,
            tc.tile_pool(name="ps_dw", bufs=2, space="PSUM") as pdw,
            tc.tile_pool(name="ps_f", bufs=4, space="PSUM") as pf,
        ):
            cst = pc.tile([128, cst_cols], BF, tag="cst", name="cst")
            nc.sync.dma_start(out=cst[:, :], in_=cstD[:, :])
            msk = pc.tile([128, 6 * WP], BF, tag="msk", name="msk")
            nc.sync.dma_start(out=msk[:, :], in_=mskD[:, :])
            cs32 = pc.tile([128, cs32_cols], F32, tag="cs32", name="cs32")
            nc.sync.dma_start(out=cs32[:, :], in_=cs32D[:, :])
            x1t = pc.tile([C, RX * W], BF, tag="x1", name="x1")

            def cv(name, r0=0, rn=None, c0=0, cn=None):
                off, w, rows = spans[name]
                rn = rows if rn is None else rn
                cn = w if cn is None else cn
                return cst[r0:r0 + rn, off + c0:off + c0 + cn]

            def cv32(name, r0=0, rn=None, c0=0, cn=None):
                off, w, rows = spans32[name]
                rn = rows if rn is None else rn
                cn = w if cn is None else cn
                return cs32[r0:r0 + rn, off + c0:off + c0 + cn]

            ones64 = cv("ones64")
            ones128 = cv("ones128")
            I128 = cv("I128")

            def chunks(N):
                c0 = 0
                while c0 < N:
                    yield c0, min(MMN, N - c0)
                    c0 += MMN

            # -- LayerNorm over channels, two skewed stages ------------------
            def ln_a(x_ap, cn, nch, ones_ap):
                ps = pps.tile([nch, MMN], F32, tag="ps", name="ps_mu")
                nc.tensor.matmul(ps[:, :cn], ones_ap, x_ap, start=True, stop=True)
                xc = pk.tile([nch, MMN], BF, tag=f"xc{nch}", name=f"xc{nch}",
                             bufs=3)
                nc.vector.tensor_sub(xc[:, :cn], x_ap, ps[:, :cn])
                x2 = pk.tile([nch, MMN], BF, tag=f"x2{nch}", name=f"x2{nch}",
                             bufs=3)
                nc.scalar.activation(x2[:, :cn], xc[:, :cn], AF.Square)
                return xc, x2

            def ln_b(st, cn, nch, ones_ap, out_xn):
                xc, x2 = st
                ps = pps.tile([nch, MMN], F32, tag="ps", name="ps_var")
                nc.tensor.matmul(ps[:, :cn], ones_ap, x2[:, :cn],
                                 start=True, stop=True)
                rs_ = pk.tile([nch, MMN], BF, tag=f"rs{nch}", name=f"rs{nch}")
                nc.scalar.activation(rs_[:, :cn], ps[:, :cn],
                                     AF.Abs_reciprocal_sqrt,
                                     bias=cv32("eps", rn=nch))
                nc.vector.tensor_mul(out_xn, xc[:, :cn], rs_[:, :cn])

            # -- ln -> padded dual-copy tile, 2-stage pipeline ---------------
            def emit_ln_pad(src_ap, Ncols, dst3):
                st = {}
                ch = list(chunks(Ncols))
                for idx in range(len(ch) + 1):
                    if idx < len(ch):
                        c0, cn = ch[idx]
                        st[idx] = ln_a(src_ap[:, c0:c0 + cn], cn, C, ones64)
                    if idx >= 1:
                        c0, cn = ch[idx - 1]
                        xn = pk.tile([C, MMN], BF, tag="xn", name="xn")
                        ln_b(st.pop(idx - 1), cn, C, ones64, xn[:, :cn])
                        r0, rn = c0 // W, cn // W
                        xn3 = xn[:, :cn].rearrange("p (r w) -> p r w", w=W)
                        nc.vector.tensor_copy(dst3[0:64, r0:r0 + rn, 1:1 + W], xn3)
                        nc.vector.tensor_copy(dst3[64:128, r0:r0 + rn, 0:W], xn3)

            def pad_tile(rows, name, tag="padt", bufs=None):
                t = pb.tile([128, rows * WP], BF, tag=tag, name=name, bufs=bufs)
                t3 = t[:, :].rearrange("p (r w) -> p r w", w=WP)
                nc.vector.memset(t3[0:64, :, 0:1], 0.0)
                nc.vector.memset(t3[0:64, :, WP - 1:WP], 0.0)
                nc.vector.memset(t3[64:128, :, WP - 2:WP], 0.0)
                return t3

            # ---------------- FSAS blocks (4-stage skewed pipeline) --------
            for ai, (s, e) in enumerate(A_BLOCKS):
                rq = e - s
                hs, he = max(s - 1, 0), min(e + 1, RX)
                rh = he - hs
                Nh, Nq = rh * W, rq * W

                xt = pio.tile([C, Nh], BF, tag="xt", name="xt")
                nc.sync.dma_start(out=xt[:, :], in_=xs[:, hs * W:he * W])
                xp3 = pad_tile(rh, "xp")
                emit_ln_pad(xt[:, :], Nh, xp3)
                if ai == 0:
                    v = xp3[:, 0:2, :].rearrange("p r w -> p (r w)")
                    nc.vector.tensor_mul(v, v, msk[:, 0:2 * WP])
                if ai == len(A_BLOCKS) - 1:
                    v = xp3[:, rh - 2:rh, :].rearrange("p r w -> p (r w)")
                    nc.vector.tensor_mul(v, v, msk[:, 2 * WP:4 * WP])

                def fs0(c0, cn):
                    r = s + c0 // W
                    out = {}
                    for m in range(3):
                        ps = pdw.tile([128, MMN], F32, tag="dw", name="ps_dw")
                        for dh in range(3):
                            ri = min(max(r - 1 + dh - hs, 0), rh - 2)
                            nc.tensor.matmul(ps[:, :cn], cv(f"QW{m}{dh}"),
                                             xp3[:, ri:ri + 2, 0:W],
                                             start=(dh == 0), stop=False)
                            nc.tensor.matmul(ps[:, :cn], cv(f"QS{m}{dh}"),
                                             xp3[0:64, ri:ri + 2, 2:2 + W],
                                             start=False, stop=(dh == 2))
                        t_ = pk.tile([128, MMN], BF, tag=f"qkv{m}",
                                     name=f"qkv{m}", bufs=4)
                        nc.scalar.activation(t_[:, :cn], ps[:, :cn], AF.Identity,
                                             bias=cv32("b_dw1", c0=m, cn=1))
                        out[m] = t_
                    return out

                def fs1(st, cn):
                    qc, kc, vc_ = st[0], st[1], st[2]
                    qT = pk.tile([128, MMN], BF, tag="qT", name="qT")
                    kT = pk.tile([128, MMN], BF, tag="kT", name="kT")
                    for src_, dst in ((qc, qT), (kc, kT)):
                        pt = pf.tile([128, MMN], BF, tag="f", name="tr")
                        for jj in range(4):
                            nc.tensor.transpose(pt[:, jj * 128:(jj + 1) * 128],
                                                src_[:, jj * 128:(jj + 1) * 128],
                                                I128)
                        nc.scalar.copy(out=dst[:, :cn], in_=pt[:, :cn])
                    ff = {}
                    for nm_, srcT, mat in (("qfr", qT, "CRbd"), ("qfi", qT, "CIbd"),
                                           ("kfr", kT, "CRbd"), ("kfi", kT, "CIbd")):
                        ps = pf.tile([128, MMN], F32, tag="f", name="ps_f")
                        nc.tensor.matmul(ps[:, :cn], cv(mat), srcT[:, :cn],
                                         start=True, stop=True)
                        t_ = pk.tile([128, MMN], BF, tag=nm_, name=nm_)
                        if nm_[0] == "k":
                            nc.scalar.copy(out=t_[:, :cn], in_=ps[:, :cn])
                        else:
                            nc.vector.tensor_copy(t_[:, :cn], ps[:, :cn])
                        ff[nm_] = t_
                    tmp = pk.tile([128, MMN], BF, tag="tmp", name="tmp")
                    pr = pk.tile([128, MMN], BF, tag="pr", name="pr", bufs=3)
                    pi = pk.tile([128, MMN], BF, tag="pi", name="pi", bufs=3)
                    nc.vector.tensor_mul(tmp[:, :cn], ff["qfr"][:, :cn],
                                         ff["kfr"][:, :cn])
                    nc.vector.tensor_mul(pr[:, :cn], ff["qfi"][:, :cn],
                                         ff["kfi"][:, :cn])
                    nc.vector.tensor_sub(pr[:, :cn], tmp[:, :cn], pr[:, :cn])
                    nc.vector.tensor_mul(tmp[:, :cn], ff["qfr"][:, :cn],
                                         ff["kfi"][:, :cn])
                    nc.vector.tensor_mul(pi[:, :cn], ff["qfi"][:, :cn],
                                         ff["kfr"][:, :cn])
                    nc.vector.tensor_add(pi[:, :cn], tmp[:, :cn], pi[:, :cn])
                    st["pr"], st["pi"] = pr, pi

                def fs2(st, cn):
                    psi = pps.tile([128, MMN], F32, tag="ps", name="ps_i")
                    nc.tensor.matmul(psi[:, :cn], cv("CRibd"), st["pr"][:, :cn],
                                     start=True, stop=False)
                    nc.tensor.matmul(psi[:, :cn], cv("CIibd"), st["pi"][:, :cn],
                                     start=False, stop=True)
                    corrT = pk.tile([128, MMN], BF, tag="corrT", name="corrT")
                    nc.vector.tensor_copy(corrT[:, :cn], psi[:, :cn])
                    pt = pf.tile([128, MMN], BF, tag="f", name="tr2")
                    for jj in range(4):
                        nc.tensor.transpose(pt[:, jj * 128:(jj + 1) * 128],
                                            corrT[:, jj * 128:(jj + 1) * 128],
                                            I128)
                    corr = pk.tile([128, MMN], BF, tag="corr", name="corr")
                    nc.scalar.copy(out=corr[:, :cn], in_=pt[:, :cn])
                    st["ln"] = ln_a(corr[:, :cn], cn, 128, ones128)

                def fs3(st, cn, c0):
                    corrn = pk.tile([128, MMN], BF, tag="corrn", name="corrn")
                    ln_b(st["ln"], cn, 128, ones128, corrn[:, :cn])
                    vcg = pk.tile([128, MMN], BF, tag="vcg", name="vcg")
                    nc.vector.scalar_tensor_tensor(
                        out=vcg[:, :cn], in0=corrn[:, :cn], scalar=cv32("g2"),
                        in1=st[2][:, :cn], op0=OP.mult, op1=OP.mult)
                    pso = pps.tile([64, MMN], F32, tag="ps", name="ps_o")
                    nc.tensor.matmul(pso[:, :cn], cv("WoT"), vcg[:, :cn],
                                     start=True, stop=not use_wob)
                    if use_wob:
                        nc.tensor.matmul(pso[:, :cn], cv("WobT"),
                                         st[2][:, :cn], start=False, stop=True)
                    xoff = (s - hs) * W + c0
                    nc.vector.scalar_tensor_tensor(
                        out=x1t[:, s * W + c0:s * W + c0 + cn], in0=pso[:, :cn],
                        scalar=cv32("b_o"), in1=xt[:, xoff:xoff + cn],
                        op0=OP.add, op1=OP.add)

                qch = list(chunks(Nq))
                S = {}
                for idx in range(len(qch) + 3):
                    if idx < len(qch):
                        S[idx] = fs0(*qch[idx])
                    if 0 <= idx - 1 < len(qch):
                        fs1(S[idx - 1], qch[idx - 1][1])
                    if 0 <= idx - 2 < len(qch):
                        fs2(S[idx - 2], qch[idx - 2][1])
                    if 0 <= idx - 3 < len(qch):
                        fs3(S.pop(idx - 3), qch[idx - 3][1], qch[idx - 3][0])

            # ---------------- DFFN: block-pairs, ln2 phase then gelu phase --
            def gs0(xq3, ys, t0, c0, cn):
                r = t0 + c0 // W
                ps1 = pdw.tile([128, MMN], F32, tag="dw", name="ps_y1")
                ps2 = pdw.tile([128, MMN], F32, tag="dw", name="ps_y2")
                for m, ps in ((0, ps1), (1, ps2)):
                    for dh in range(3):
                        ri = r - 1 + dh - ys
                        nc.tensor.matmul(ps[:, :cn], cv(f"YW{m}{dh}"),
                                         xq3[:, ri:ri + 2, 0:W],
                                         start=(dh == 0), stop=False)
                        nc.tensor.matmul(ps[:, :cn], cv(f"YS{m}{dh}"),
                                         xq3[0:64, ri:ri + 2, 2:2 + W],
                                         start=False, stop=(dh == 2))
                g1 = pk.tile([128, MMN], BF, tag="g1", name="g1")
                nc.scalar.activation(g1[:, :cn], ps1[:, :cn], AF.Gelu,
                                     bias=cv32("b_dw2", c0=0, cn=1))
                gp = pk.tile([128, MMN], BF, tag="gp", name="gp", bufs=3)
                nc.vector.scalar_tensor_tensor(
                    out=gp[:, :cn], in0=ps2[:, :cn],
                    scalar=cv32("b_dw2", c0=1, cn=1), in1=g1[:, :cn],
                    op0=OP.add, op1=OP.mult)
                return gp

            def gs1(gp, t0, c0, cn):
                pso = pps.tile([64, MMN], F32, tag="ps", name="ps_o2")
                nc.tensor.matmul(pso[:, :cn], cv("W2T"), gp[:, :cn],
                                 start=True, stop=True)
                outc = pio.tile([C, MMN], F32, tag="outt", name="outt", bufs=2)
                nc.vector.scalar_tensor_tensor(
                    out=outc[:, :cn], in0=pso[:, :cn], scalar=cv32("b2o"),
                    in1=x1t[:, t0 * W + c0:t0 * W + c0 + cn],
                    op0=OP.add, op1=OP.add)
                oc = (t0 - 2) * W + c0
                nc.sync.dma_start(out=outD[:, oc:oc + cn], in_=outc[:, :cn])

            xqs = []
            for bi_, (t0, u0) in enumerate(B_BLOCKS):
                ys, ye = t0 - 1, u0 + 1
                ry = ye - ys
                xq3 = pad_tile(ry, f"xq{bi_}", tag=f"xq{bi_}", bufs=1)
                xqs.append(xq3)
                emit_ln_pad(x1t[:, ys * W:ye * W], ry * W, xq3)
                if bi_ == 0:
                    v = xq3[:, 0:1, :].rearrange("p r w -> p (r w)")
                    nc.vector.tensor_mul(v, v, msk[:, 4 * WP:5 * WP])
                if bi_ == len(B_BLOCKS) - 1:
                    v = xq3[:, ry - 1:ry, :].rearrange("p r w -> p (r w)")
                    nc.vector.tensor_mul(v, v, msk[:, 5 * WP:6 * WP])

            work = []
            for bi_, (t0, u0) in enumerate(B_BLOCKS):
                for c0, cn in chunks((u0 - t0) * W):
                    work.append((bi_, t0, c0, cn))
            G = {}
            for idx in range(len(work) + 1):
                if idx < len(work):
                    bi_, t0, c0, cn = work[idx]
                    G[idx] = gs0(xqs[bi_], t0 - 1, t0, c0, cn)
                if idx >= 1:
                    bi_, t0, c0, cn = work[idx - 1]
                    gs1(G.pop(idx - 1), t0, c0, cn)

    nc.compile()
    return nc


# ---------------------------------------------------------------- entry point

def _wire_ntff_hook():
    try:
        import antenv.axon_hooks  # noqa: F401
        return
    except ImportError:
        pass
    mod = types.ModuleType("antenv.axon_hooks")
    holder = [None]
    mod.set_axon_ntff_profile_hook = lambda h: holder.__setitem__(0, h)
    mod.get_axon_ntff_profile_hook = lambda: holder[0]
    sys.modules["antenv.axon_hooks"] = mod
    try:
        from trn_agent_boot import trn_boot
        hook = trn_boot._ntff_profile_via_ctypes("/opt/axon/libaxon_pjrt.so")
        mod.set_axon_ntff_profile_hook(hook)
    except Exception:
        pass


def _run_device(args):
    global _LAST_EXEC_NS
    import ml_dtypes
    bf16 = ml_dtypes.bfloat16
    import concourse.bass as bass
    import concourse.bacc as bacc
    import concourse.mybir as mybir
    from concourse import tile
    from concourse.bass_utils import run_bass_kernel_spmd

    _wire_ntff_hook()
    # enable walrus LDWEIGHTS optimization (background weight-buffer loads):
    # without it every matmul serializes behind a full-array weight load,
    # which also keeps the PE activity monitor from ever unthrottling.

    (cst, spans, cs32, spans32), xs_list, msk_list, use_wob, zb = \
        _prepare_host(args, bf16)
    nc = _build(spans, cst.shape[1], spans32, cs32.shape[1], use_wob, zb,
                mybir, bacc, tile, bass)

    in_maps = [{"xs": xs_list[i], "cst": cst, "cs32": cs32, "msk": msk_list[i]}
               for i in range(NCORES)]
    res = run_bass_kernel_spmd(nc, in_maps, list(range(NCORES)), trace=True)
    global _LAST_RES
    _LAST_RES = res
    if res.exec_time_ns:
        _LAST_EXEC_NS = res.exec_time_ns

    out = np.empty((B, C, H, W), np.float32)
    for core in range(NCORES):
        bi, si = core // 4, core % 4
        o = np.asarray(res.results[core]["out"], np.float32)
        out[bi, :, 64 * si:64 * (si + 1), :] = o.reshape(C, RS, W)
    return out


# ------------------------------------------------------------- host fallback

def _conv1x1(x, w, b):
    Bn, Cn, Hn, Wn = x.shape
    y = np.matmul(w.astype(np.float32), x.reshape(Bn, Cn, Hn * Wn))
    return y.reshape(Bn, w.shape[0], Hn, Wn) + b[None, :, None, None]


def _dwconv3(x, w, b):
    Bn, Cn, Hn, Wn = x.shape
    xp = np.pad(x, ((0, 0), (0, 0), (1, 1), (1, 1)))
    y = np.zeros_like(x)
    for dh in range(3):
        for dw in range(3):
            y += w[:, 0, dh, dw][None, :, None, None] * xp[:, :, dh:dh + Hn, dw:dw + Wn]
    return y + b[None, :, None, None]


def _ln_ch(x, g, b):
    mu = x.mean(axis=1, keepdims=True)
    var = ((x - mu) ** 2).mean(axis=1, keepdims=True)
    return (x - mu) / np.sqrt(var + EPS) * g[None, :, None, None] + b[None, :, None, None]


def _patches(x):
    b, c, h, w = x.shape
    return x.reshape(b, c, h // P, w // P, P, P)


def _unpatch(x):
    b, c, hp, wp, _, _ = x.shape
    return x.reshape(b, c, hp * P, wp * P)


def _gelu(x):
    from scipy.special import erf
    return 0.5 * x * (1.0 + erf(x / np.float32(np.sqrt(2.0))))


def _host_reference(a):
    x = a["x"]
    h = _conv1x1(_ln_ch(x, a["ln1_g"], a["ln1_b"]), a["att_hid_w"], a["att_hid_b"])
    hq = _dwconv3(h, a["att_dw_w"], a["att_dw_b"])
    Cq = hq.shape[1] // 3
    q, k, v = hq[:, :Cq], hq[:, Cq:2 * Cq], hq[:, 2 * Cq:]
    qf = np.fft.rfft2(_patches(q))
    kf = np.fft.rfft2(_patches(k))
    corr = np.fft.irfft2(qf * kf, s=(P, P)).astype(np.float32)
    corr = _ln_ch(_unpatch(corr), a["att_norm_g"], a["att_norm_b"])
    x1 = x + _conv1x1(v * corr, a["att_out_w"], a["att_out_b"])
    y = _conv1x1(_ln_ch(x1, a["ln2_g"], a["ln2_b"]), a["ffn_in_w"], a["ffn_in_b"])
    yf = np.fft.rfft2(_patches(y)) * a["ffn_fft"]
    y = _unpatch(np.fft.irfft2(yf, s=(P, P)).astype(np.float32))
    yd = _dwconv3(y, a["ffn_dw_w"], a["ffn_dw_b"])
    Hh = yd.shape[1] // 2
    return x1 + _conv1x1(_gelu(yd[:, :Hh]) * yd[:, Hh:], a["ffn_out_w"], a["ffn_out_b"])


def kernel(x, ln1_g, ln1_b, att_hid_w, att_hid_b, att_dw_w, att_dw_b,
           att_norm_g, att_norm_b, att_out_w, att_out_b,
           ln2_g, ln2_b, ffn_in_w, ffn_in_b, ffn_fft,
           ffn_dw_w, ffn_dw_b, ffn_out_w, ffn_out_b):
    args = {k: np.asarray(v, dtype=np.float32) for k, v in locals().items()}
    try:
        return _run_device(args)
    except Exception as e:  # pragma: no cover - device unavailable
        import traceback
        traceback.print_exc()
        sys.stderr.write(f"[kernel] device path failed ({e!r}); host fallback\n")
        return _host_reference(args).astype(np.float32)

